# revision 33
# baseline (speedup 1.0000x reference)
"""CBAM attention Trainium2 kernel.

Full inputs: x [32, 256, 64, 64], w1 [16, 256], w2 [256, 16], ws [1, 2, 7, 7].
Data-parallel across 8 NeuronCores: 4 samples per core, weights replicated.

Per-core layout: channels on partitions (2 tiles of 128 per sample), spatial
(64*64=4096) along the free dim.  x is loaded via casting DMAs straight to
bf16; pooling runs as 4x-mode tensor_scalar ops with add/max accumulators;
everything downstream (channel attention scale, channel-sum matmul, transposes
for the channel max, 7x7 conv as im2col matmul, spatial gate multiply) stays
in bf16, and the f32 output is produced by casting DMAs on the way out.
"""

import numpy as np

B, C, H, W = 32, 256, 64, 64
NCORES = 8
B_LOC = B // NCORES          # 4 samples per core
MID = 16
HW = H * W                   # 4096
NT = C // 128                # 2 channel tiles
PW = 70                      # padded row width (W + 2*3)
FROW = PW * 71               # featpad row size (padded plane 4900 + spare 70)
PROW = PW * PW               # 4900 padded plane size
IROW = 64 * PW               # im2col row size (64 y-rows of stride-70 data)

_cached_nc = None


def _build(reps=1, cast_in=True, gp_off=False):
    from concourse import bass, bacc, tile
    import concourse.mybir as mybir

    F32 = mybir.dt.float32
    BF16 = mybir.dt.bfloat16
    AF = mybir.ActivationFunctionType
    ALU = mybir.AluOpType

    nc = bacc.Bacc("TRN2", target_bir_lowering=False, debug=False,
                   num_devices=NCORES)

    x_d = nc.dram_tensor("x", [B_LOC, C, H, W], F32, kind="ExternalInput")
    ident_d = nc.dram_tensor("ident", [128, 128], BF16, kind="ExternalInput")
    ones_d = nc.dram_tensor("ones", [128, 1], BF16, kind="ExternalInput")
    w1t_d = nc.dram_tensor("w1t", [128, NT, MID], F32, kind="ExternalInput")
    w1ta_d = nc.dram_tensor("w1ta", [128, NT, MID], F32,
                            kind="ExternalInput")
    w2t_d = nc.dram_tensor("w2t", [MID, C], F32, kind="ExternalInput")
    wconv_d = nc.dram_tensor("wconv", [98, 128], BF16, kind="ExternalInput")
    out_d = nc.dram_tensor("out", [B_LOC, C, H, W], F32, kind="ExternalOutput")

    with tile.TileContext(nc) as tc:
        with (
            tc.tile_pool(name="xs", bufs=1) as xpool,
            tc.tile_pool(name="work", bufs=1) as work,
            tc.tile_pool(name="imc", bufs=2) as imcpool,
            tc.tile_pool(name="grep", bufs=2) as gpool,
            tc.tile_pool(name="sm", bufs=2) as smpool,
            tc.tile_pool(name="tp", bufs=2, space="PSUM") as tppool,
            tc.tile_pool(name="p4", bufs=2, space="PSUM") as p4pool,
            tc.tile_pool(name="gps", bufs=2, space="PSUM") as gpspool,
            tc.tile_pool(name="mlp", bufs=2, space="PSUM") as mlppool,
        ):
            # ---- constants ----
            ident = work.tile([128, 128], BF16, tag="ident")
            ones = work.tile([128, 1], BF16, tag="ones")
            w1t = work.tile([128, NT, MID], F32, tag="w1t")
            w1ta = work.tile([128, NT, MID], F32, tag="w1ta")
            w2t = work.tile([MID, C], F32, tag="w2t")
            wconv = work.tile([98, 128], BF16, tag="wconv")
            nc.sync.dma_start(ident[:], ident_d.ap())
            nc.sync.dma_start(ones[:], ones_d.ap())
            nc.sync.dma_start(w1t[:], w1t_d.ap())
            nc.sync.dma_start(w1ta[:], w1ta_d.ap())
            nc.sync.dma_start(w2t[:], w2t_d.ap())
            nc.sync.dma_start(wconv[:], wconv_d.ap())

            # ---- working buffers ----
            featpads = [work.tile([2, FROW], BF16, tag=f"featpad{i}",
                                  name=f"featpad{i}") for i in range(2)]
            feat_dxs = [work.tile([14, PROW], BF16, tag=f"feat_dx{i}",
                                  name=f"feat_dx{i}") for i in range(2)]

            for i in range(2):
                fv = featpads[i][:].rearrange("p (y x) -> p y x", y=71, x=PW)
                nc.vector.memset(fv[:, 0:3, :], 0.0)
                nc.vector.memset(fv[:, 67:71, :], 0.0)
                nc.vector.memset(fv[:, 3:67, 0:3], 0.0)
                nc.vector.memset(fv[:, 3:67, 67:70], 0.0)

            for rep in range(reps):
              # stat cols: 2*(t*4+b) = sum, 2*(t*4+b)+1 = max ; 16 + t*4+b = ca
              stat = work.tile([128, 24], F32, tag="stat", bufs=2,
                               name=f"stat{rep}")
              hs = work.tile([MID, 3 * B_LOC], F32, tag="hs", bufs=2,
                             name=f"hs{rep}")
              xbf = [xpool.tile([128, NT, HW], BF16, tag="xb",
                                name=f"xb{rep}_{b}",
                                bufs=4 if cast_in else 3)
                     for b in range(B_LOC)]
              if not cast_in:
                  xsf = [xpool.tile([128, NT, HW], F32, tag="xsf",
                                    name=f"xsf{rep}_{b}", bufs=2)
                         for b in range(B_LOC)]
              xm2 = [xpool.tile([128, HW], BF16, tag="xm2",
                                name=f"xm2{rep}_{b}", bufs=2)
                     for b in range(B_LOC)]
              for b in range(B_LOC):
                  featpad = featpads[b % 2]
                  feat_dx = feat_dxs[b % 2]
                  fpv = featpad[:].rearrange("p (y x) -> p y x", y=71, x=PW)
                  # ---- load x + pooling ----
                  src = x_d.ap()[b].rearrange("ch h w -> ch (h w)").rearrange(
                      "(t c) e -> c t e", t=NT, c=128)
                  if cast_in:
                      # casting DMA (f32 -> bf16); sum+max on DVE (4x mode)
                      for t in range(NT):
                          nc.gpsimd.dma_start(xbf[b][:, t:t + 1, :],
                                              src[:, t:t + 1, :])
                      for t in range(NT):
                          j = t * B_LOC + b
                          nc.vector.tensor_scalar(
                              xbf[b][:, t, :], xbf[b][:, t, :], 1.0, None,
                              op0=ALU.mult, op1=ALU.add,
                              accum_out=stat[:, 2 * j:2 * j + 1])
                          nc.vector.tensor_scalar(
                              xbf[b][:, t, :], xbf[b][:, t, :], 1.0, None,
                              op0=ALU.mult, op1=ALU.max,
                              accum_out=stat[:, 2 * j + 1:2 * j + 2])
                  else:
                      # f32 HWDGE load; ACT cast+sum; DVE max
                      for t in range(NT):
                          nc.sync.dma_start(xsf[b][:, t:t + 1, :],
                                            src[:, t:t + 1, :])
                      for t in range(NT):
                          j = t * B_LOC + b
                          nc.scalar.activation(
                              xbf[b][:, t, :], xsf[b][:, t, :], AF.Copy,
                              accum_out=stat[:, 2 * j:2 * j + 1])
                          nc.vector.tensor_scalar(
                              xbf[b][:, t, :], xbf[b][:, t, :], 1.0, None,
                              op0=ALU.mult, op1=ALU.max,
                              accum_out=stat[:, 2 * j + 1:2 * j + 2])

                  # ---- MLP (1/HW folded into w1ta for the sum column) ----
                  hp = mlppool.tile([MID, 2], F32, tag="mlp",
                                    name=f"hp{rep}_{b}")
                  for t in range(NT):
                      j = t * B_LOC + b
                      nc.tensor.matmul(
                          hp[:, 0:1], w1ta[:, t, :],
                          stat[:, 2 * j:2 * j + 1],
                          start=(t == 0), stop=(t == NT - 1))
                  for t in range(NT):
                      j = t * B_LOC + b
                      nc.tensor.matmul(
                          hp[:, 1:2], w1t[:, t, :],
                          stat[:, 2 * j + 1:2 * j + 2],
                          start=(t == 0), stop=(t == NT - 1))
                  hsum = hs[:, 2 * B_LOC + b:2 * B_LOC + b + 1]
                  nc.scalar.activation(hs[:, 2 * b:2 * b + 2], hp[:],
                                       AF.Relu, accum_out=hsum)
                  for t in range(NT):
                      op = mlppool.tile([128, 1], F32, tag="mlp",
                                        name=f"op{rep}_{b}_{t}")
                      nc.tensor.matmul(
                          op[:], w2t[:, t * 128:(t + 1) * 128],
                          hsum, start=True, stop=True)
                      cacol = stat[:, 16 + t * B_LOC + b:17 + t * B_LOC + b]
                      nc.scalar.activation(cacol, op[:], AF.Sigmoid)

                  # ---- xc = x * ca in bf16 (DVE 4x) ----
                  for t in range(NT):
                      cacol = stat[:, 16 + t * B_LOC + b:17 + t * B_LOC + b]
                      nc.vector.tensor_scalar_mul(
                          xbf[b][:, t, :], xbf[b][:, t, :], cacol)

                  # ---- channel sum (PE bf16) -> feat row 0 (ACT evict) ----
                  for jc in range(8):
                      p4 = p4pool.tile([1, 512], F32, tag="p4",
                                       name=f"p4_{rep}_{b}_{jc}")
                      for t in range(NT):
                          nc.tensor.matmul(
                              p4[:], ones[:],
                              xbf[b][:, t, jc * 512:(jc + 1) * 512],
                              start=(t == 0), stop=(t == NT - 1))
                      dst = fpv[0:1, 3 + 8 * jc:3 + 8 * jc + 8, 3:3 + W]
                      nc.scalar.activation(
                          dst, p4[0:1, :].rearrange("p (y x) -> p y x",
                                                    y=8, x=W),
                          AF.Copy)

                  # ---- channel max: tile-pair max, transpose, reduce ----
                  # boost the last sample's spine: it gates the out-drain
                  import contextlib
                  prio = (tc.high_priority(offset=200)
                          if b == B_LOC - 1 else contextlib.nullcontext())
                  with prio:
                      nc.vector.tensor_max(xm2[b][:], xbf[b][:, 0, :],
                                           xbf[b][:, 1, :])
                      samax = smpool.tile([128, 32], BF16, tag="samax",
                                          name=f"samax{rep}_{b}")
                      for g in range(8):
                          tp = tppool.tile([128, 4, 128], BF16, tag="tp",
                                           name=f"tp{rep}_{b}_{g}")
                          for cc in range(4):
                              nc.tensor.transpose(
                                  tp[:, cc, :],
                                  xm2[b][:, (4 * g + cc) * 128:
                                         (4 * g + cc + 1) * 128],
                                  ident[:])
                          nc.vector.reduce_max(
                              samax[:, 4 * g:4 * g + 4], tp[:],
                              axis=mybir.AxisListType.X)
                      # [128, 32] -> [32, 128] so rows reach the free axis
                      smp = mlppool.tile([32, 128], BF16, tag="mlp",
                                         name=f"smp{rep}_{b}")
                      nc.tensor.transpose(smp[:], samax[:], ident[:])
                      samaxT = smpool.tile([32, 128], BF16, tag="samaxT",
                                           name=f"samaxT{rep}_{b}")
                      nc.scalar.activation(samaxT[:], smp[:], AF.Copy)
                      # scatter into feat row 1 (hw = k*128+e ; y = 2k+e//64)
                      for y1 in range(2):
                          nc.sync.dma_start(
                              fpv[1:2, 3 + y1:3 + y1 + 64:2, 3:3 + W],
                              samaxT[:, y1 * W:(y1 + 1) * W])

                  # ---- im2col stage A: dx shifts, one DMA per c ----
                  fph = featpad[:].tensor
                  for c in range(2):
                      nc.scalar.dma_start(
                          feat_dx[c * 7:(c + 1) * 7, :],
                          bass.AP(fph, c * FROW,
                                  [[FROW, 1], [1, 7], [1, PROW]]))
                  # ---- im2col stage B: dy windows, one DMA per c ----
                  imc = imcpool.tile([98, IROW], BF16, tag="imc",
                                     name=f"imc{rep}_{b}")
                  fdh = feat_dx[:].tensor
                  for c in range(2):
                      nc.scalar.dma_start(
                          imc[c * 49:(c + 1) * 49, :],
                          bass.AP(fdh, c * 7 * PROW,
                                  [[PROW, 7], [PW, 7], [1, IROW]]))

                  # ---- conv (PE bf16) + sigmoid (ACT) + gate mul (DVE) ----
                  imv = imc[:].rearrange("p (y x) -> p y x", y=64, x=PW)
                  dst_d = out_d.ap()[b].rearrange(
                      "ch h w -> ch (h w)").rearrange(
                      "(t c) e -> c t e", t=NT, c=128)
                  for hh in range(2):
                      grep = gpool.tile([128, HW // 2], BF16, tag="grep",
                                        name=f"grep{rep}_{b}_{hh}")
                      for jc in range(4):
                          gp = gpspool.tile([128, 512], F32, tag="gps",
                                            name=f"gps{rep}_{b}_{hh}_{jc}")
                          y0 = (hh * 4 + jc) * 8
                          nc.tensor.matmul(
                              gp[:], wconv[:], imv[:, y0:y0 + 8, 0:W],
                              start=True, stop=True)
                          nc.scalar.activation(
                              grep[:, jc * 512:(jc + 1) * 512], gp[:],
                              AF.Sigmoid)
                      sl = slice(hh * (HW // 2), (hh + 1) * (HW // 2))
                      for t in range(NT):
                          nc.vector.tensor_mul(
                              xbf[b][:, t, sl], xbf[b][:, t, sl], grep[:])
                      # store this half (casting DMA bf16 -> f32)
                      for t in range(NT):
                          nc.gpsimd.dma_start(dst_d[:, t:t + 1, sl],
                                              xbf[b][:, t:t + 1, sl])

    nc.compile()
    return nc


def _host_consts(w1, w2, ws):
    import ml_dtypes
    bf16 = ml_dtypes.bfloat16
    ident = np.eye(128, dtype=np.float32).astype(bf16)
    ones = np.ones((128, 1), np.float32).astype(bf16)
    # w1 [MID, C] -> lhsT layout [128, NT, MID]
    w1t = np.ascontiguousarray(
        np.asarray(w1, np.float32).T.reshape(NT, 128, MID).transpose(
            1, 0, 2)).astype(np.float32)
    w1ta = (w1t / float(HW)).astype(np.float32)
    w2t = np.ascontiguousarray(np.asarray(w2, np.float32).T)
    wf = np.asarray(ws, np.float32)[0]                       # [2, 7, 7]
    # row order k = c*49 + dx*7 + dy (stage-B iterates dx over partitions)
    wcol = np.empty((98, 1), np.float32)
    for c in range(2):
        scale = 1.0 / C if c == 0 else 1.0
        for dx in range(7):
            for dy in range(7):
                wcol[c * 49 + dx * 7 + dy, 0] = wf[c, dy, dx] * scale
    wconv = np.repeat(wcol, 128, axis=1).astype(bf16)
    return ident, ones, w1t, w1ta, w2t, wconv


def kernel(x, w1, w2, ws):
    global _cached_nc
    from concourse.bass_utils import run_bass_kernel_spmd

    if _cached_nc is None:
        _cached_nc = _build()
    nc = _cached_nc

    x = np.asarray(x, np.float32)
    ident, ones, w1t, w1ta, w2t, wconv = _host_consts(w1, w2, ws)
    in_maps = []
    for i in range(NCORES):
        in_maps.append({
            "x": np.ascontiguousarray(x[i * B_LOC:(i + 1) * B_LOC]),
            "ident": ident, "ones": ones, "w1t": w1t, "w1ta": w1ta,
            "w2t": w2t, "wconv": wconv,
        })
    res = run_bass_kernel_spmd(nc, in_maps, core_ids=list(range(NCORES)))
    out = np.concatenate([res.results[i]["out"] for i in range(NCORES)],
                         axis=0)
    return out.astype(np.float32)
